# revision 1
# baseline (speedup 1.0000x reference)
"""Trainium2 Bass kernel for nn_CrossOrganismAttention.

Data-parallel over 8 cores (batch sharded). Per core, rows are processed in
tiles of 128 batch elements (= 384 (b,s) rows). Main activation stream is
feature-major (features on partitions) for matmuls; attention, layernorm
stats and pooling run row-major (batch on partitions), converted via PE
transposes. Matmuls use float32r (1 cy/row at N>=256) for the f32 stream and
bf16 for attention/FFN weights.
"""

import numpy as np

import concourse.bass as bass
import concourse.mybir as mybir
from concourse.bacc import Bacc
from concourse.tile import TileContext
from concourse.bass_utils import run_bass_kernel_spmd

B, S, D, H, DH = 65536, 3, 256, 4, 64
NCORES = 8
NB = 128  # batch elements per tile
EPS = 1e-5
F32 = mybir.dt.float32
F32R = mybir.dt.float32r
BF16 = mybir.dt.bfloat16
AF = mybir.ActivationFunctionType
OP = mybir.AluOpType
AX = mybir.AxisListType
GELU = AF.Gelu  # debug hook: sim lacks Gelu; tests may swap in Tanh


def _chunk_lhst(w_t: np.ndarray) -> np.ndarray:
    """(K, M) lhsT -> (128, K//128 * M) with chunk c at cols [c*M:(c+1)*M]."""
    k, m = w_t.shape
    assert k % 128 == 0
    return np.ascontiguousarray(
        w_t.reshape(k // 128, 128, m).transpose(1, 0, 2).reshape(128, -1)
    )


def _per_part(v: np.ndarray) -> np.ndarray:
    """(n*128,) bias -> (128, n) with chunk c in col c."""
    n = v.shape[0] // 128
    return np.ascontiguousarray(v.reshape(n, 128).T)


# Packed constant layouts: (key, cols). Offsets are cumulative per pack.
_PACK_F32R = [("c_spw2t", 512), ("c_inwt", 1536)]
_PACK_F32 = [("c_spec3", 6), ("c_spb1", 2), ("c_ffn1b", 4), ("c_x2b", 2),
             ("c_g2b", 256), ("c_b2b", 256), ("c_id", 128)]
_PACK_BF16 = [("c_outwt", 512), ("c_ffn1t", 1024), ("c_ffn2t", 1024),
              ("c_g1d", 256), ("c_idb", 128)]
_PACK_SMALL = [("c_w1", 256), ("c_qkvb", 768), ("c_ones", 128)]


def host_consts(p: dict) -> dict:
    """Precompute packed weight/bias constants from the reference params."""
    f = lambda x: np.asarray(x, np.float32)
    sq = 1.0 / np.sqrt(DH)
    in_w, in_b = f(p["in_w"]), f(p["in_b"])
    out_w, out_b = f(p["out_w"]), f(p["out_b"])
    sp_w1, sp_b1 = f(p["sp_w1"]), f(p["sp_b1"])
    sp_w2, sp_b2 = f(p["sp_w2"]), f(p["sp_b2"])
    ffn_w1, ffn_b1 = f(p["ffn_w1"]), f(p["ffn_b1"])
    ffn_w2, ffn_b2 = f(p["ffn_w2"]), f(p["ffn_b2"])
    ln1_g, ln1_b = f(p["ln1_g"]), f(p["ln1_b"])
    ln2_g, ln2_b = f(p["ln2_g"]), f(p["ln2_b"])
    species = f(p["species_emb"])

    # tokens'' = emb + species + sp_b2 + out_b ("species3"); fm layout per chunk.
    sp3 = species + sp_b2[None, :] + out_b[None, :]  # (3, 256)
    spec3 = np.ascontiguousarray(
        sp3.T.reshape(2, 128, 3).transpose(1, 0, 2).reshape(128, 6)
    )  # (128, 2*3): chunk c at cols [3c:3c+3]

    # qkv weights: lhsT = tokens-chunk (stationary), moving rhs = in_w.T.
    inwt = in_w.T.copy()  # (256, 768)
    inwt[:, :D] *= sq
    qkvb = in_b - in_w @ out_b  # compensate out_b folded into tokens''
    qkvb[:D] *= sq

    consts = {
        "c_spec3": spec3,
        "c_w1": np.ascontiguousarray(sp_w1[:, 0][None, :]),  # (1, 256)
        "c_spb1": _per_part(sp_b1),  # (128, 2)
        "c_spw2t": _chunk_lhst(sp_w2.T),  # (128, 2*256)
        "c_inwt": _chunk_lhst(inwt),  # (128, 2*768)
        "c_qkvb": np.ascontiguousarray(qkvb[None, :]),  # (1, 768)
        "c_outwt": _chunk_lhst(out_w.T).astype(np.float32),  # (128, 2*256)
        "c_ffn1t": _chunk_lhst((ffn_w1 * ln1_g[None, :]).T),  # (128, 2*512)
        "c_ffn1b": _per_part(ffn_b1 + ffn_w1 @ ln1_b),  # (128, 4)
        "c_ffn2t": _chunk_lhst(ffn_w2.T),  # (128, 4*256)
        "c_g1d": _chunk_lhst(np.diag(ln1_g))[
            :, [c * 256 + c * 128 + j for c in range(2) for j in range(128)]
        ],  # (128, 2*128): diag block c
        "c_x2b": _per_part(ln1_b + ffn_b2),  # (128, 2)
        "c_g2b": np.ascontiguousarray(np.tile(ln2_g[None, :], (128, 1))),
        "c_b2b": np.ascontiguousarray(np.tile(ln2_b[None, :], (128, 1))),
        "c_id": np.eye(128, dtype=np.float32),
        "c_ones": np.ones((1, 128), np.float32),
    }
    import ml_dtypes

    for k in ("c_outwt", "c_ffn1t", "c_ffn2t", "c_g1d"):
        consts[k] = consts[k].astype(ml_dtypes.bfloat16).view(np.uint16)
    consts["c_idb"] = np.eye(128).astype(ml_dtypes.bfloat16).view(np.uint16)

    def pack(layout):
        return np.ascontiguousarray(
            np.concatenate([consts[k].astype(consts[k].dtype) for k, _ in layout], axis=1)
        )

    return {
        "c_pf32r": pack(_PACK_F32R).astype(np.float32),
        "c_pf32": pack(_PACK_F32).astype(np.float32),
        "c_pbf16": pack(_PACK_BF16),
        "c_small": pack(_PACK_SMALL).astype(np.float32),
    }


def host_inputs(core: int, inputs: dict, consts: dict) -> dict:
    bl = B // NCORES
    b0 = core * bl
    ntiles = bl // NB
    emb = np.ascontiguousarray(np.asarray(inputs["organism_embeddings"], np.float32)[b0 : b0 + bl])
    a = np.asarray(inputs["anomaly_scores"], np.float32)[b0 : b0 + bl]
    # per tile: (s, b) order for fm columns
    a_t = np.ascontiguousarray(
        a.reshape(ntiles, NB, S).transpose(0, 2, 1).reshape(ntiles, S * NB)
    )
    mask = np.asarray(inputs["organism_mask"], bool)[b0 : b0 + bl]
    valid = (~mask).astype(np.float32)
    w = valid / valid.sum(axis=1, keepdims=True)
    wt = np.ascontiguousarray(w.T)  # (3, bl)
    mbt = np.ascontiguousarray((mask.astype(np.float32) * -1e9).T)  # (3, bl)
    m = {"emb": emb, "a": a_t, "wt": wt, "mbt": mbt}
    m.update(consts)
    return m


def build(bl: int, stage=None) -> bass.Bass:
    ntiles = bl // NB
    nc = Bacc()

    EMB = nc.declare_dram_parameter("emb", [bl, S, D], F32, isOutput=False)
    A = nc.declare_dram_parameter("a", [ntiles, S * NB], F32R, isOutput=False)
    WT = nc.declare_dram_parameter("wt", [S, bl], F32, isOutput=False)
    MBT = nc.declare_dram_parameter("mbt", [S, bl], F32, isOutput=False)
    OUT = nc.declare_dram_parameter("out", [bl, D], F32, isOutput=True)

    packs = {
        "c_pf32r": (_PACK_F32R, F32R, 128),
        "c_pf32": (_PACK_F32, F32, 128),
        "c_pbf16": (_PACK_BF16, BF16, 128),
        "c_small": (_PACK_SMALL, F32R, 1),
    }
    cparams = {}
    for pk, (layout, dt, rows) in packs.items():
        w = sum(n for _, n in layout)
        pdt = mybir.dt.uint16 if dt == BF16 else dt
        cparams[pk] = nc.declare_dram_parameter(pk, [rows, w], pdt, isOutput=False)

    with TileContext(nc) as tc:
        with (
            tc.tile_pool(name="consts", bufs=1) as cpool,
            tc.tile_pool(name="pin", bufs=3) as pin,
            tc.tile_pool(name="sb", bufs=2) as sb,
            tc.tile_pool(name="sbs", bufs=3) as sbs,
            tc.tile_pool(name="pout", bufs=3) as pout,
            tc.tile_pool(name="psA", bufs=4, space="PSUM") as psA,
            tc.tile_pool(name="psQ", bufs=2, space="PSUM") as psQ,
            tc.tile_pool(name="psB", bufs=2, space="PSUM") as psB,
        ):
            C = {}
            for pk, (layout, dt, rows) in packs.items():
                w = sum(n for _, n in layout)
                ct = cpool.tile([rows, w], dt, tag=pk)
                src = cparams[pk][:, :]
                if dt == BF16:
                    src = src.bitcast(BF16)
                nc.sync.dma_start(ct[:], src)
                off = 0
                for k, n in layout:
                    C[k] = ct[:, off : off + n]
                    off += n
            # PE warm-up: touch each const pack once on PE so no later
            # matmul needs more than one fresh semaphore wait (the LW
            # struct has a single wait slot).
            wps = psA.tile([128, 384], F32, tag="psA")
            nc.tensor.transpose(wps[:, 0:128], C["c_id"], C["c_id"])
            nc.tensor.matmul(wps[:, 0:128],
                             C["c_spw2t"][:, 0:128], C["c_spw2t"][:, 0:128])
            nc.tensor.matmul(wps[:, 0:256], C["c_ones"], C["c_w1"])
            wpb = psB.tile([128, 384], BF16, tag="psB")
            nc.tensor.transpose(wpb[:, 0:128], C["c_idb"], C["c_idb"])
            # DVE/ACT warm-up: observe each const-pack DMA semaphore once.
            wsc = sbs.tile([128, 8], F32, tag="wsc")
            nc.vector.tensor_copy(wsc[:, 0:1], C["c_id"][:, 0:1])
            nc.vector.tensor_copy(wsc[:, 1:2], C["c_idb"][:, 0:1])
            nc.vector.tensor_copy(wsc[:, 2:3], C["c_spw2t"][:, 0:1])
            nc.vector.tensor_copy(wsc[0:1, 3:4], C["c_w1"][0:1, 0:1])
            nc.scalar.activation(wsc[:, 4:5], C["c_id"][:, 1:2], AF.Copy)
            nc.scalar.activation(wsc[:, 5:6], C["c_idb"][:, 1:2], AF.Copy)
            nc.scalar.activation(wsc[:, 6:7], C["c_spw2t"][:, 1:2], AF.Copy)
            nc.scalar.activation(wsc[0:1, 7:8], C["c_w1"][0:1, 1:2], AF.Copy)
            for t in range(ntiles):
                _tile_body(nc, tc, C, pin, sb, sbs, pout, psA, psQ, psB,
                           EMB, A, WT, MBT, OUT, t, stage)
    nc.compile()
    return nc


def _dbg(nc, OUT, B0, ap):
    nc.sync.dma_start(OUT[B0 : B0 + NB, :], ap)


def _tile_body(nc, tc, C, pin, sb, sbs, pout, psA, psQ, psB,
               EMB, A, WT, MBT, OUT, t, stage=None):
    B0 = t * NB
    mm = nc.tensor.matmul

    # ---- input DMAs ----
    emb_rm = pin.tile([128, 768], F32, tag="emb_rm")
    nc.sync.dma_start(emb_rm[:].rearrange("p (s d) -> p s d", s=3), EMB[B0 : B0 + NB])
    a_t = pin.tile([1, 384], F32R, tag="a_t")
    nc.sync.dma_start(a_t[:], A[t : t + 1, :])
    w_t = pin.tile([3, 128], F32, tag="w_t")
    nc.sync.dma_start(w_t[:], WT[:, B0 : B0 + NB])
    mb_t = pin.tile([3, 128], F32, tag="mb_t")
    nc.sync.dma_start(mb_t[:], MBT[:, B0 : B0 + NB])

    idf = C["c_id"]
    idb = C["c_idb"]

    # ---- emb -> fm (+species/sp_b2/out_b) ----
    emb2 = sb.tile([128, 768], BF16, tag="emb2")  # (c, s*128+b)
    for c in range(2):
        ps = psA.tile([128, 384], F32, tag="psA")
        for s in range(3):
            nc.tensor.transpose(
                ps[:, s * 128 : s * 128 + 128],
                emb_rm[:, s * 256 + c * 128 : s * 256 + c * 128 + 128],
                idf,
            )
        nc.vector.tensor_tensor(
            out=emb2[:, c * 384 : (c + 1) * 384].rearrange("p (s b) -> p s b", s=3),
            in0=ps[:].rearrange("p (s b) -> p s b", s=3),
            in1=C["c_spec3"][:, c * 3 : c * 3 + 3][:, :, None].broadcast_to((128, 3, 128)),
            op=OP.add,
        )

    if stage == 1:
        return _dbg(nc, OUT, B0, emb2[:, 0:512].bitcast(F32))

    # ---- score MLP: h = gelu(a * w1 + b1) ----
    h2 = sb.tile([128, 768], F32R, tag="h2")
    for c in range(2):
        ps = psA.tile([128, 384], F32, tag="psA")
        mm(ps[:], C["c_w1"][0:1, c * 128 : c * 128 + 128],
           a_t[:])
        nc.scalar.activation(h2[:, c * 384 : (c + 1) * 384], ps[:], GELU,
                             bias=C["c_spb1"][:, c : c + 1])

    if stage == 2:
        return _dbg(nc, OUT, B0, h2[:, 0:256].bitcast(F32))

    # ---- tokens = emb2 + sp_w2 @ h ----
    tok2 = sb.tile([128, 768], F32R, tag="tok2")
    for c in range(2):
        ps = psA.tile([128, 384], F32, tag="psA")
        for kc in range(2):
            mm(ps[:],
               C["c_spw2t"][:, kc * 256 + c * 128 : kc * 256 + c * 128 + 128],
               h2[:, kc * 384 : (kc + 1) * 384],
               start=(kc == 0), stop=(kc == 1))
        nc.vector.tensor_tensor(out=tok2[:, c * 384 : (c + 1) * 384],
                                in0=ps[:], in1=emb2[:, c * 384 : (c + 1) * 384],
                                op=OP.add)

    if stage == 3:
        return _dbg(nc, OUT, B0, tok2[:, 0:256].bitcast(F32))

    # ---- qkv (row-major out, per s) ----
    qkv3 = sb.tile([128, 2304], BF16, tag="qkv3")  # (s, 768)
    for s in range(3):
        for nh in range(2):
            ps = psA.tile([128, 384], F32, tag="psA")
            for kc in range(2):
                mm(ps[:],
                   tok2[:, kc * 384 + s * 128 : kc * 384 + s * 128 + 128],
                   C["c_inwt"][:, kc * 768 + nh * 384 : kc * 768 + (nh + 1) * 384],
                   start=(kc == 0), stop=False)
            mm(ps[:],
               C["c_ones"],
               C["c_qkvb"][0:1, nh * 384 : (nh + 1) * 384],
               start=False, stop=True)
            nc.scalar.activation(
                qkv3[:, s * 768 + nh * 384 : s * 768 + (nh + 1) * 384],
                ps[:], AF.Copy)

    if stage == 4:
        return _dbg(nc, OUT, B0, qkv3[:, 0:512].bitcast(F32))

    # ---- mask columns -> row-major ----
    psm = psA.tile([128, 384], F32, tag="psA")
    nc.tensor.transpose(psm[:, 0:3], w_t[:], idf[0:3, 0:3])
    nc.tensor.transpose(psm[:, 3:6], mb_t[:], idf[0:3, 0:3])
    wm = sbs.tile([128, 6], F32, tag="wm")
    nc.vector.tensor_copy(wm[:], psm[:, 0:6])

    if stage == 5:
        return _dbg(nc, OUT, B0, emb_rm[:, 0:256])

    # ---- attention scores ----
    qv = qkv3[:].rearrange("p (s f) -> p s f", s=3)
    prod = sb.tile([128, 2304], BF16, tag="prod")
    nc.vector.tensor_tensor(
        out=prod[:].rearrange("p (q k f) -> p q k f", q=3, k=3),
        in0=qv[:, :, None, 0:256].broadcast_to((128, 3, 3, 256)),
        in1=qv[:, None, :, 256:512].broadcast_to((128, 3, 3, 256)),
        op=OP.mult,
    )
    att = sbs.tile([128, 36], BF16, tag="att")  # (q, k, h)
    with nc.allow_low_precision("bf16 attention scores"):
        nc.vector.tensor_reduce(
            out=att[:].rearrange("p (q k h) -> p q k h", q=3, k=3),
            in_=prod[:].rearrange("p (q k h e) -> p q k h e", q=3, k=3, h=4),
            axis=AX.X, op=OP.add,
        )
    attv = att[:].rearrange("p (q k h) -> p q k h", q=3, k=3)
    attm = sbs.tile([128, 36], F32, tag="attm")
    amv = attm[:].rearrange("p (q k h) -> p q k h", q=3, k=3)
    nc.vector.tensor_tensor(
        out=amv, in0=attv,
        in1=wm[:, 3:6][:, None, :, None].broadcast_to((128, 3, 3, 4)),
        op=OP.add,
    )
    mx = sbs.tile([128, 12], F32, tag="mx")
    mxv = mx[:].rearrange("p (q h) -> p q h", q=3)
    nc.vector.tensor_tensor(out=mxv, in0=amv[:, :, 0], in1=amv[:, :, 1], op=OP.max)
    nc.vector.tensor_tensor(out=mxv, in0=mxv, in1=amv[:, :, 2], op=OP.max)
    es = sbs.tile([128, 36], F32, tag="es")
    esv = es[:].rearrange("p (q k h) -> p q k h", q=3, k=3)
    nc.vector.tensor_tensor(
        out=esv, in0=amv,
        in1=mxv[:, :, None, :].broadcast_to((128, 3, 3, 4)), op=OP.subtract,
    )
    nc.scalar.activation(es[:], es[:], AF.Exp)
    den = sbs.tile([128, 12], F32, tag="den")
    dv = den[:].rearrange("p (q h) -> p q h", q=3)
    nc.vector.tensor_tensor(out=dv, in0=esv[:, :, 0], in1=esv[:, :, 1], op=OP.add)
    nc.vector.tensor_tensor(out=dv, in0=dv, in1=esv[:, :, 2], op=OP.add)
    rden = sbs.tile([128, 12], F32, tag="rden")
    nc.vector.reciprocal(rden[:], den[:])
    p = sbs.tile([128, 36], F32, tag="p")
    pv = p[:].rearrange("p (q k h) -> p q k h", q=3, k=3)
    nc.vector.tensor_tensor(
        out=pv, in0=esv,
        in1=rden[:].rearrange("p (q h) -> p q h", q=3)[:, :, None, :].broadcast_to((128, 3, 3, 4)),
        op=OP.mult,
    )

    if stage == 6:
        return _dbg(nc, OUT, B0, prod[:, 0:512].bitcast(F32))

    # ---- mix: o_q = v0 + p1*(v1-v0) + p2*(v2-v0) ----
    d1 = sbs.tile([128, 256], BF16, tag="d1")
    d2 = sbs.tile([128, 256], BF16, tag="d2")
    nc.vector.tensor_tensor(out=d1[:], in0=qv[:, 1, 512:768], in1=qv[:, 0, 512:768], op=OP.subtract)
    nc.vector.tensor_tensor(out=d2[:], in0=qv[:, 2, 512:768], in1=qv[:, 0, 512:768], op=OP.subtract)
    t1 = sb.tile([128, 768], BF16, tag="t1")
    t2 = sb.tile([128, 768], BF16, tag="t2")
    for dd, tt_, k in ((d1, t1, 1), (d2, t2, 2)):
        nc.vector.tensor_tensor(
            out=tt_[:].rearrange("p (q h e) -> p q h e", q=3, h=4),
            in0=dd[:, None, :].broadcast_to((128, 3, 256)).rearrange("p q (h e) -> p q h e", h=4),
            in1=pv[:, :, k, :, None].broadcast_to((128, 3, 4, 64)),
            op=OP.mult,
        )
    o = sb.tile([128, 768], BF16, tag="o")
    nc.vector.tensor_tensor(out=o[:], in0=t1[:], in1=t2[:], op=OP.add)
    nc.vector.tensor_tensor(
        out=o[:].rearrange("p (q f) -> p q f", q=3),
        in0=o[:].rearrange("p (q f) -> p q f", q=3),
        in1=qv[:, 0:1, 512:768].broadcast_to((128, 3, 256)),
        op=OP.add,
    )

    if stage == 7:
        return _dbg(nc, OUT, B0, o[:, 0:512].bitcast(F32))

    # ---- o -> fm; out-proj; x1 = tokens'' + o @ out_w.T ----
    ofm = sb.tile([128, 768], BF16, tag="ofm")
    for c in range(2):
        ps = psB.tile([128, 384], BF16, tag="psB")
        for q in range(3):
            nc.tensor.transpose(
                ps[:, q * 128 : q * 128 + 128],
                o[:, q * 256 + c * 128 : q * 256 + c * 128 + 128],
                idb,
            )
        nc.vector.tensor_copy(ofm[:, c * 384 : (c + 1) * 384], ps[:])
    x1f = sb.tile([128, 768], BF16, tag="x1f")
    for c in range(2):
        ps = psA.tile([128, 384], F32, tag="psA")
        for kc in range(2):
            mm(ps[:],
               C["c_outwt"][:, kc * 256 + c * 128 : kc * 256 + c * 128 + 128],
               ofm[:, kc * 384 : (kc + 1) * 384],
               start=(kc == 0), stop=(kc == 1))
        nc.vector.tensor_tensor(out=x1f[:, c * 384 : (c + 1) * 384], in0=ps[:],
                                in1=tok2[:, c * 384 : (c + 1) * 384], op=OP.add)

    if stage == 8:
        return _dbg(nc, OUT, B0, x1f[:, 0:512].bitcast(F32))

    # ---- LN1 (row-major) ----
    n1rm = _layernorm_rm(nc, tc, sb, sbs, psQ, psA, x1f, idb, "1")

    if stage == 9:
        return _dbg(nc, OUT, B0, n1rm[:, 0:512].bitcast(F32))

    # ---- n1 -> fm ----
    n1f = sb.tile([128, 768], BF16, tag="n1f")
    for c in range(2):
        ps = psB.tile([128, 384], BF16, tag="psB")
        for s in range(3):
            nc.tensor.transpose(
                ps[:, s * 128 : s * 128 + 128],
                n1rm[:, s * 256 + c * 128 : s * 256 + c * 128 + 128],
                idb,
            )
        nc.vector.tensor_copy(n1f[:, c * 384 : (c + 1) * 384], ps[:])

    if stage == 10:
        return _dbg(nc, OUT, B0, n1f[:, 0:512].bitcast(F32))

    # ---- FFN ----
    f1 = sb.tile([128, 1536], BF16, tag="f1")
    for oc in range(4):
        ps = psA.tile([128, 384], F32, tag="psA")
        for kc in range(2):
            mm(ps[:],
               C["c_ffn1t"][:, kc * 512 + oc * 128 : kc * 512 + oc * 128 + 128],
               n1f[:, kc * 384 : (kc + 1) * 384],
               start=(kc == 0), stop=(kc == 1))
        nc.scalar.activation(f1[:, oc * 384 : (oc + 1) * 384], ps[:], GELU,
                             bias=C["c_ffn1b"][:, oc : oc + 1])
    x2f = sb.tile([128, 768], BF16, tag="x2f")
    for c in range(2):
        ps = psA.tile([128, 384], F32, tag="psA")
        for kc in range(4):
            mm(ps[:],
               C["c_ffn2t"][:, kc * 256 + c * 128 : kc * 256 + c * 128 + 128],
               f1[:, kc * 384 : (kc + 1) * 384],
               start=(kc == 0), stop=False)
        mm(ps[:], C["c_g1d"][:, c * 128 : (c + 1) * 128],
           n1f[:, c * 384 : (c + 1) * 384], start=False, stop=True)
        nc.scalar.activation(x2f[:, c * 384 : (c + 1) * 384], ps[:], AF.Identity,
                             bias=C["c_x2b"][:, c : c + 1])

    if stage == 11:
        return _dbg(nc, OUT, B0, x2f[:, 0:512].bitcast(F32))

    # ---- LN2 (row-major) ----
    n2rm = _layernorm_rm(nc, tc, sb, sbs, psQ, psA, x2f, idb, "2")

    if stage == 12:
        return _dbg(nc, OUT, B0, n2rm[:, 0:512].bitcast(F32))

    # ---- masked mean pool + ln2 affine ----
    acc = pout.tile([128, 256], F32, tag="acc")
    nc.vector.scalar_tensor_tensor(
        out=acc[:], in0=n2rm[:, 0:256], scalar=wm[:, 0:1], in1=n2rm[:, 0:256],
        op0=OP.mult, op1=OP.bypass,
    )
    for s in (1, 2):
        nc.vector.scalar_tensor_tensor(
            out=acc[:], in0=n2rm[:, s * 256 : (s + 1) * 256],
            scalar=wm[:, s : s + 1], in1=acc[:], op0=OP.mult, op1=OP.add,
        )
    outt = pout.tile([128, 256], F32, tag="outt")
    nc.vector.tensor_tensor(out=outt[:], in0=acc[:], in1=C["c_g2b"], op=OP.mult)
    nc.vector.tensor_tensor(out=outt[:], in0=outt[:], in1=C["c_b2b"], op=OP.add)
    nc.sync.dma_start(OUT[B0 : B0 + NB, :], outt[:])


LN_STAGE = None


def _layernorm_rm(nc, tc, sb, sbs, psQ, psA, xf, idb, suffix):
    """xf: (128, 2*384) bf16 feature-major -> normalized row-major (128, 768)
    bf16 (no affine)."""
    psr = psQ.tile([128, 768], BF16, tag="psQ")
    for c in range(2):
        for s in range(3):
            nc.tensor.transpose(
                psr[:, s * 256 + c * 128 : s * 256 + c * 128 + 128],
                xf[:, c * 384 + s * 128 : c * 384 + s * 128 + 128],
                idb,
            )
    xrm = sb.tile([128, 768], BF16, tag="xrm" + suffix)
    sums = sbs.tile([128, 3], F32, tag="sums" + suffix)
    sqs = sbs.tile([128, 3], F32, tag="sqs" + suffix)
    scr = sb.tile([128, 768], BF16, tag="scr" + suffix)
    for s in range(3):
        nc.scalar.activation(
            xrm[:, s * 256 : (s + 1) * 256], psr[:, s * 256 : (s + 1) * 256],
            AF.Copy, accum_out=sums[:, s : s + 1],
        )
        nc.scalar.activation(
            scr[:, s * 256 : (s + 1) * 256], xrm[:, s * 256 : (s + 1) * 256],
            AF.Square, accum_out=sqs[:, s : s + 1],
        )
    if LN_STAGE in (81, 815, 82):
        return xrm
    mean = sbs.tile([128, 3], F32, tag="mean" + suffix)
    nc.vector.tensor_scalar_mul(mean[:], sums[:], 1.0 / 256.0)
    ex2 = sbs.tile([128, 3], F32, tag="ex2" + suffix)
    nc.vector.tensor_scalar(out=ex2[:], in0=sqs[:], scalar1=1.0 / 256.0,
                            scalar2=EPS, op0=OP.mult, op1=OP.add)
    m2 = sbs.tile([128, 3], F32, tag="m2" + suffix)
    nc.vector.tensor_tensor(out=m2[:], in0=mean[:], in1=mean[:], op=OP.mult)
    var = sbs.tile([128, 3], F32, tag="var" + suffix)
    nc.vector.tensor_tensor(out=var[:], in0=ex2[:], in1=m2[:], op=OP.subtract)
    sd = sbs.tile([128, 3], F32, tag="sd" + suffix)
    nc.scalar.activation(sd[:], var[:], AF.Sqrt)
    rstd = sbs.tile([128, 3], F32, tag="rstd" + suffix)
    nc.vector.reciprocal(rstd[:], sd[:])
    mb = sbs.tile([128, 3], F32, tag="mb" + suffix)
    nc.vector.scalar_tensor_tensor(out=mb[:], in0=mean[:], scalar=-1.0,
                                   in1=rstd[:], op0=OP.mult, op1=OP.mult)
    if LN_STAGE == 83:
        return xrm
    nrm = sb.tile([128, 768], BF16, tag="nrm" + suffix)
    for s in range(3):
        nc.scalar.activation(
            nrm[:, s * 256 : (s + 1) * 256], xrm[:, s * 256 : (s + 1) * 256],
            AF.Identity, bias=mb[:, s : s + 1], scale=rstd[:, s : s + 1],
        )
    return nrm


_CACHE: dict = {}


def _get_nc(bl: int) -> bass.Bass:
    if bl not in _CACHE:
        _CACHE[bl] = build(bl)
    return _CACHE[bl]


def kernel(**inputs) -> np.ndarray:
    consts = host_consts(inputs)
    nc = _get_nc(B // NCORES)
    in_maps = [host_inputs(i, inputs, consts) for i in range(NCORES)]
    res = run_bass_kernel_spmd(nc, in_maps, core_ids=list(range(NCORES)))
    return np.concatenate([r["out"] for r in res.results], axis=0)



# revision 5
# speedup vs baseline: 1.1663x; 1.1663x over previous
"""Trainium2 Bass kernel for nn_CrossOrganismAttention.

Data-parallel over 8 cores (batch sharded). Per core, rows are processed in
tiles of 128 batch elements (= 384 (b,s) rows). Main activation stream is
feature-major (features on partitions) for matmuls; attention, layernorm
stats and pooling run row-major (batch on partitions), converted via PE
transposes. Matmuls use float32r (1 cy/row at N>=256) for the f32 stream and
bf16 for attention/FFN weights.
"""

import numpy as np

import concourse.bass as bass
import concourse.mybir as mybir
from concourse.bacc import Bacc
from concourse.tile import TileContext
from concourse.bass_utils import run_bass_kernel_spmd

B, S, D, H, DH = 65536, 3, 256, 4, 64
NCORES = 8
NB = 128  # batch elements per tile
EPS = 1e-5
F32 = mybir.dt.float32
F32R = mybir.dt.float32r
BF16 = mybir.dt.bfloat16
AF = mybir.ActivationFunctionType
OP = mybir.AluOpType
AX = mybir.AxisListType
GELU = AF.Gelu_apprx_tanh  # tanh-approx: same ACT table set as Tanh
I32 = mybir.dt.int32
RSQRT_MAGIC = 0x5F3759DF


def _chunk_lhst(w_t: np.ndarray) -> np.ndarray:
    """(K, M) lhsT -> (128, K//128 * M) with chunk c at cols [c*M:(c+1)*M]."""
    k, m = w_t.shape
    assert k % 128 == 0
    return np.ascontiguousarray(
        w_t.reshape(k // 128, 128, m).transpose(1, 0, 2).reshape(128, -1)
    )


def _per_part(v: np.ndarray) -> np.ndarray:
    """(n*128,) bias -> (128, n) with chunk c in col c."""
    n = v.shape[0] // 128
    return np.ascontiguousarray(v.reshape(n, 128).T)


# Packed constant layouts: (key, cols). Offsets are cumulative per pack.
_PACK_F32R = [("c_spw2t", 512), ("c_inwt", 1536)]
_PACK_F32 = [("c_spec3", 6), ("c_spb1", 2), ("c_ffn1b", 4), ("c_x2b", 2),
             ("c_g2b", 256), ("c_b2b", 256), ("c_id", 128)]
_PACK_BF16 = [("c_outwt", 512), ("c_ffn1t", 1024), ("c_ffn2t", 1024),
              ("c_g1d", 256), ("c_idb", 128)]
_PACK_SMALL = [("c_w1", 256), ("c_qkvb", 768), ("c_ones", 128)]


def host_consts(p: dict) -> dict:
    """Precompute packed weight/bias constants from the reference params."""
    f = lambda x: np.asarray(x, np.float32)
    sq = 1.0 / np.sqrt(DH)
    in_w, in_b = f(p["in_w"]), f(p["in_b"])
    out_w, out_b = f(p["out_w"]), f(p["out_b"])
    sp_w1, sp_b1 = f(p["sp_w1"]), f(p["sp_b1"])
    sp_w2, sp_b2 = f(p["sp_w2"]), f(p["sp_b2"])
    ffn_w1, ffn_b1 = f(p["ffn_w1"]), f(p["ffn_b1"])
    ffn_w2, ffn_b2 = f(p["ffn_w2"]), f(p["ffn_b2"])
    ln1_g, ln1_b = f(p["ln1_g"]), f(p["ln1_b"])
    ln2_g, ln2_b = f(p["ln2_g"]), f(p["ln2_b"])
    species = f(p["species_emb"])

    # tokens'' = emb + species + sp_b2 + out_b ("species3"); fm layout per chunk.
    sp3 = species + sp_b2[None, :] + out_b[None, :]  # (3, 256)
    spec3 = np.ascontiguousarray(
        sp3.T.reshape(2, 128, 3).transpose(1, 0, 2).reshape(128, 6)
    )  # (128, 2*3): chunk c at cols [3c:3c+3]

    # qkv weights: lhsT = tokens-chunk (stationary), moving rhs = in_w.T.
    inwt = in_w.T.copy()  # (256, 768)
    inwt[:, :D] *= sq
    qkvb = in_b - in_w @ out_b  # compensate out_b folded into tokens''
    qkvb[:D] *= sq

    consts = {
        "c_spec3": spec3,
        "c_w1": np.ascontiguousarray(sp_w1[:, 0][None, :]),  # (1, 256)
        "c_spb1": _per_part(sp_b1),  # (128, 2)
        "c_spw2t": _chunk_lhst(sp_w2.T),  # (128, 2*256)
        "c_inwt": _chunk_lhst(inwt),  # (128, 2*768)
        "c_qkvb": np.ascontiguousarray(qkvb[None, :]),  # (1, 768)
        "c_outwt": _chunk_lhst(out_w.T).astype(np.float32),  # (128, 2*256)
        "c_ffn1t": _chunk_lhst((ffn_w1 * ln1_g[None, :]).T),  # (128, 2*512)
        "c_ffn1b": _per_part(ffn_b1 + ffn_w1 @ ln1_b),  # (128, 4)
        "c_ffn2t": _chunk_lhst(ffn_w2.T),  # (128, 4*256)
        "c_g1d": _chunk_lhst(np.diag(ln1_g))[
            :, [c * 256 + c * 128 + j for c in range(2) for j in range(128)]
        ],  # (128, 2*128): diag block c
        "c_x2b": _per_part(ln1_b + ffn_b2),  # (128, 2)
        "c_g2b": np.ascontiguousarray(np.tile(ln2_g[None, :], (128, 1))),
        "c_b2b": np.ascontiguousarray(np.tile(ln2_b[None, :], (128, 1))),
        "c_id": np.eye(128, dtype=np.float32),
        "c_ones": np.ones((1, 128), np.float32),
    }
    import ml_dtypes

    for k in ("c_outwt", "c_ffn1t", "c_ffn2t", "c_g1d"):
        consts[k] = consts[k].astype(ml_dtypes.bfloat16).view(np.uint16)
    consts["c_idb"] = np.eye(128).astype(ml_dtypes.bfloat16).view(np.uint16)

    def pack(layout):
        return np.ascontiguousarray(
            np.concatenate([consts[k].astype(consts[k].dtype) for k, _ in layout], axis=1)
        )

    return {
        "c_pf32r": pack(_PACK_F32R).astype(np.float32),
        "c_pf32": pack(_PACK_F32).astype(np.float32),
        "c_pbf16": pack(_PACK_BF16),
        "c_small": pack(_PACK_SMALL).astype(np.float32),
    }


def host_inputs(core: int, inputs: dict, consts: dict) -> dict:
    bl = B // NCORES
    b0 = core * bl
    ntiles = bl // NB
    emb = np.ascontiguousarray(np.asarray(inputs["organism_embeddings"], np.float32)[b0 : b0 + bl])
    a = np.asarray(inputs["anomaly_scores"], np.float32)[b0 : b0 + bl]
    # per tile: (s, b) order for fm columns
    a_t = np.ascontiguousarray(
        a.reshape(ntiles, NB, S).transpose(0, 2, 1).reshape(ntiles, S * NB)
    )
    mask = np.asarray(inputs["organism_mask"], bool)[b0 : b0 + bl]
    valid = (~mask).astype(np.float32)
    w = valid / valid.sum(axis=1, keepdims=True)
    wt = np.ascontiguousarray(w.T)  # (3, bl)
    mbt = np.ascontiguousarray((mask.astype(np.float32) * -1e9).T)  # (3, bl)
    m = {"emb": emb, "a": a_t, "wt": wt, "mbt": mbt}
    m.update(consts)
    return m


def build(bl: int, stage=None) -> bass.Bass:
    ntiles = bl // NB
    nc = Bacc()

    EMB = nc.declare_dram_parameter("emb", [bl, S, D], F32, isOutput=False)
    A = nc.declare_dram_parameter("a", [ntiles, S * NB], F32R, isOutput=False)
    WT = nc.declare_dram_parameter("wt", [S, bl], F32, isOutput=False)
    MBT = nc.declare_dram_parameter("mbt", [S, bl], F32, isOutput=False)
    OUT = nc.declare_dram_parameter("out", [bl, D], F32, isOutput=True)

    packs = {
        "c_pf32r": (_PACK_F32R, F32R, 128),
        "c_pf32": (_PACK_F32, F32, 128),
        "c_pbf16": (_PACK_BF16, BF16, 128),
        "c_small": (_PACK_SMALL, F32R, 1),
    }
    cparams = {}
    for pk, (layout, dt, rows) in packs.items():
        w = sum(n for _, n in layout)
        pdt = mybir.dt.uint16 if dt == BF16 else dt
        cparams[pk] = nc.declare_dram_parameter(pk, [rows, w], pdt, isOutput=False)

    with TileContext(nc) as tc:
        with (
            tc.tile_pool(name="consts", bufs=1) as cpool,
            tc.tile_pool(name="pin", bufs=3) as pin,
            tc.tile_pool(name="sb", bufs=2) as sb,
            tc.tile_pool(name="sbs", bufs=3) as sbs,
            tc.tile_pool(name="pout", bufs=3) as pout,
            tc.tile_pool(name="psA", bufs=4, space="PSUM") as psA,
            tc.tile_pool(name="psQ", bufs=2, space="PSUM") as psQ,
            tc.tile_pool(name="psB", bufs=2, space="PSUM") as psB,
        ):
            C = {}
            for pk, (layout, dt, rows) in packs.items():
                w = sum(n for _, n in layout)
                ct = cpool.tile([rows, w], dt, tag=pk)
                src = cparams[pk][:, :]
                if dt == BF16:
                    src = src.bitcast(BF16)
                nc.sync.dma_start(ct[:], src)
                off = 0
                for k, n in layout:
                    C[k] = ct[:, off : off + n]
                    off += n
            # PE warm-up: touch each const pack once on PE so no later
            # matmul needs more than one fresh semaphore wait (the LW
            # struct has a single wait slot).
            wps = psA.tile([128, 384], F32, tag="psA")
            nc.tensor.transpose(wps[:, 0:128], C["c_id"], C["c_id"])
            nc.tensor.matmul(wps[:, 0:128],
                             C["c_spw2t"][:, 0:128], C["c_spw2t"][:, 0:128])
            nc.tensor.matmul(wps[:, 0:256], C["c_ones"], C["c_w1"])
            wpb = psB.tile([128, 384], BF16, tag="psB")
            nc.tensor.transpose(wpb[:, 0:128], C["c_idb"], C["c_idb"])
            # DVE/ACT warm-up: observe each const-pack DMA semaphore once.
            wsc = sbs.tile([128, 8], F32, tag="wsc")
            nc.vector.tensor_copy(wsc[:, 0:1], C["c_id"][:, 0:1])
            nc.vector.tensor_copy(wsc[:, 1:2], C["c_idb"][:, 0:1])
            nc.vector.tensor_copy(wsc[:, 2:3], C["c_spw2t"][:, 0:1])
            nc.vector.tensor_copy(wsc[0:1, 3:4], C["c_w1"][0:1, 0:1])
            nc.scalar.activation(wsc[:, 4:5], C["c_id"][:, 1:2], AF.Copy)
            nc.scalar.activation(wsc[:, 5:6], C["c_idb"][:, 1:2], AF.Copy)
            nc.scalar.activation(wsc[:, 6:7], C["c_spw2t"][:, 1:2], AF.Copy)
            nc.scalar.activation(wsc[0:1, 7:8], C["c_w1"][0:1, 1:2], AF.Copy)
            for t in range(ntiles):
                _tile_body(nc, tc, C, pin, sb, sbs, pout, psA, psQ, psB,
                           EMB, A, WT, MBT, OUT, t, stage)
    nc.compile()
    return nc


def _dbg(nc, OUT, B0, ap):
    nc.sync.dma_start(OUT[B0 : B0 + NB, :], ap)


def _tile_body(nc, tc, C, pin, sb, sbs, pout, psA, psQ, psB,
               EMB, A, WT, MBT, OUT, t, stage=None):
    B0 = t * NB
    mm = nc.tensor.matmul

    # ---- input DMAs ----
    emb_rm = pin.tile([128, 768], F32, tag="emb_rm")
    nc.sync.dma_start(emb_rm[:].rearrange("p (s d) -> p s d", s=3), EMB[B0 : B0 + NB])
    a_t = pin.tile([1, 384], F32R, tag="a_t")
    nc.sync.dma_start(a_t[:], A[t : t + 1, :])
    w_t = pin.tile([3, 128], F32, tag="w_t")
    nc.sync.dma_start(w_t[:], WT[:, B0 : B0 + NB])
    mb_t = pin.tile([3, 128], F32, tag="mb_t")
    nc.sync.dma_start(mb_t[:], MBT[:, B0 : B0 + NB])

    idf = C["c_id"]
    idb = C["c_idb"]

    # ---- emb -> fm (+species/sp_b2/out_b) ----
    emb2 = sb.tile([128, 768], BF16, tag="emb2")  # (c, s*128+b)
    for c in range(2):
        ps = psA.tile([128, 384], F32, tag="psA")
        for s in range(3):
            nc.tensor.transpose(
                ps[:, s * 128 : s * 128 + 128],
                emb_rm[:, s * 256 + c * 128 : s * 256 + c * 128 + 128],
                idf,
            )
        nc.vector.tensor_tensor(
            out=emb2[:, c * 384 : (c + 1) * 384].rearrange("p (s b) -> p s b", s=3),
            in0=ps[:].rearrange("p (s b) -> p s b", s=3),
            in1=C["c_spec3"][:, c * 3 : c * 3 + 3][:, :, None].broadcast_to((128, 3, 128)),
            op=OP.add,
        )

    if stage == 1:
        return _dbg(nc, OUT, B0, emb2[:, 0:512].bitcast(F32))

    # ---- score MLP: h = gelu(a * w1 + b1) ----
    h2 = sb.tile([128, 768], F32R, tag="h2")
    for c in range(2):
        ps = psA.tile([128, 384], F32, tag="psA")
        mm(ps[:], C["c_w1"][0:1, c * 128 : c * 128 + 128],
           a_t[:])
        nc.scalar.activation(h2[:, c * 384 : (c + 1) * 384], ps[:], GELU,
                             bias=C["c_spb1"][:, c : c + 1])

    if stage == 2:
        return _dbg(nc, OUT, B0, h2[:, 0:256].bitcast(F32))

    # ---- tokens = emb2 + sp_w2 @ h ----
    tok2 = sb.tile([128, 768], F32R, tag="tok2")
    for c in range(2):
        ps = psA.tile([128, 384], F32, tag="psA")
        for kc in range(2):
            mm(ps[:],
               C["c_spw2t"][:, kc * 256 + c * 128 : kc * 256 + c * 128 + 128],
               h2[:, kc * 384 : (kc + 1) * 384],
               start=(kc == 0), stop=(kc == 1))
        nc.vector.tensor_tensor(out=tok2[:, c * 384 : (c + 1) * 384],
                                in0=ps[:], in1=emb2[:, c * 384 : (c + 1) * 384],
                                op=OP.add)

    if stage == 3:
        return _dbg(nc, OUT, B0, tok2[:, 0:256].bitcast(F32))

    # ---- qkv (row-major out, per s) ----
    qkv3 = sb.tile([128, 2304], BF16, tag="qkv3")  # (s, 768)
    for s in range(3):
        for nh in range(2):
            ps = psA.tile([128, 384], F32, tag="psA")
            for kc in range(2):
                mm(ps[:],
                   tok2[:, kc * 384 + s * 128 : kc * 384 + s * 128 + 128],
                   C["c_inwt"][:, kc * 768 + nh * 384 : kc * 768 + (nh + 1) * 384],
                   start=(kc == 0), stop=False)
            mm(ps[:],
               C["c_ones"],
               C["c_qkvb"][0:1, nh * 384 : (nh + 1) * 384],
               start=False, stop=True)
            nc.scalar.activation(
                qkv3[:, s * 768 + nh * 384 : s * 768 + (nh + 1) * 384],
                ps[:], AF.Copy)

    if stage == 4:
        return _dbg(nc, OUT, B0, qkv3[:, 0:512].bitcast(F32))

    # ---- mask columns -> row-major ----
    psm = psA.tile([128, 384], F32, tag="psA")
    nc.tensor.transpose(psm[:, 0:3], w_t[:], idf[0:3, 0:3])
    nc.tensor.transpose(psm[:, 3:6], mb_t[:], idf[0:3, 0:3])
    wm = sbs.tile([128, 6], F32, tag="wm")
    nc.vector.tensor_copy(wm[:], psm[:, 0:6])

    if stage == 5:
        return _dbg(nc, OUT, B0, emb_rm[:, 0:256])

    # ---- attention scores ----
    qv = qkv3[:].rearrange("p (s f) -> p s f", s=3)
    prod = sb.tile([128, 2304], BF16, tag="prod")
    nc.vector.tensor_tensor(
        out=prod[:].rearrange("p (q k f) -> p q k f", q=3, k=3),
        in0=qv[:, :, None, 0:256].broadcast_to((128, 3, 3, 256)),
        in1=qv[:, None, :, 256:512].broadcast_to((128, 3, 3, 256)),
        op=OP.mult,
    )
    att = sbs.tile([128, 36], BF16, tag="att")  # (q, k, h)
    with nc.allow_low_precision("bf16 attention scores"):
        nc.vector.tensor_reduce(
            out=att[:].rearrange("p (q k h) -> p q k h", q=3, k=3),
            in_=prod[:].rearrange("p (q k h e) -> p q k h e", q=3, k=3, h=4),
            axis=AX.X, op=OP.add,
        )
    attv = att[:].rearrange("p (q k h) -> p q k h", q=3, k=3)
    attm = sbs.tile([128, 36], F32, tag="attm")
    amv = attm[:].rearrange("p (q k h) -> p q k h", q=3, k=3)
    nc.vector.tensor_tensor(
        out=amv, in0=attv,
        in1=wm[:, 3:6][:, None, :, None].broadcast_to((128, 3, 3, 4)),
        op=OP.add,
    )
    mx = sbs.tile([128, 12], F32, tag="mx")
    mxv = mx[:].rearrange("p (q h) -> p q h", q=3)
    nc.vector.tensor_tensor(out=mxv, in0=amv[:, :, 0], in1=amv[:, :, 1], op=OP.max)
    nc.vector.tensor_tensor(out=mxv, in0=mxv, in1=amv[:, :, 2], op=OP.max)
    es = sbs.tile([128, 36], F32, tag="es")
    esv = es[:].rearrange("p (q k h) -> p q k h", q=3, k=3)
    nc.vector.tensor_tensor(
        out=esv, in0=amv,
        in1=mxv[:, :, None, :].broadcast_to((128, 3, 3, 4)), op=OP.subtract,
    )
    # softmax via tanh (stays in the gelu_apprx_tanh ACT table set):
    # t = tanh(v/2); e^v = (1+t)/(1-t); p_i = n_i / sum(n) with
    # n_i = (1+t_i) * prod_{j!=i} (1-t_j). v<=0 after max-sub, so 1-t >= 1.
    tth = sbs.tile([128, 36], F32, tag="tth")
    nc.scalar.activation(tth[:], es[:], AF.Tanh, scale=0.5)
    up = sbs.tile([128, 36], F32, tag="up")
    nc.vector.tensor_scalar(out=up[:], in0=tth[:], scalar1=1.0, scalar2=None,
                            op0=OP.add)
    wm1 = sbs.tile([128, 36], F32, tag="wm1")
    nc.vector.tensor_scalar(out=wm1[:], in0=tth[:], scalar1=-1.0, scalar2=1.0,
                            op0=OP.mult, op1=OP.add)
    wv = wm1[:].rearrange("p (q k h) -> p q k h", q=3, k=3)
    aa = sbs.tile([128, 36], F32, tag="aa")
    av = aa[:].rearrange("p (q k h) -> p q k h", q=3, k=3)
    nc.vector.tensor_tensor(out=av[:, :, 0], in0=wv[:, :, 1], in1=wv[:, :, 2], op=OP.mult)
    nc.vector.tensor_tensor(out=av[:, :, 1], in0=wv[:, :, 0], in1=wv[:, :, 2], op=OP.mult)
    nc.vector.tensor_tensor(out=av[:, :, 2], in0=wv[:, :, 0], in1=wv[:, :, 1], op=OP.mult)
    nn_ = sbs.tile([128, 36], F32, tag="nn_")
    nv = nn_[:].rearrange("p (q k h) -> p q k h", q=3, k=3)
    nc.vector.tensor_tensor(out=nv, in0=up[:].rearrange("p (q k h) -> p q k h", q=3, k=3),
                            in1=av, op=OP.mult)
    den = sbs.tile([128, 12], F32, tag="den")
    dv = den[:].rearrange("p (q h) -> p q h", q=3)
    nc.vector.tensor_tensor(out=dv, in0=nv[:, :, 0], in1=nv[:, :, 1], op=OP.add)
    nc.vector.tensor_tensor(out=dv, in0=dv, in1=nv[:, :, 2], op=OP.add)
    rden = sbs.tile([128, 12], F32, tag="rden")
    nc.vector.reciprocal(rden[:], den[:])
    p = sbs.tile([128, 36], F32, tag="p")
    pv = p[:].rearrange("p (q k h) -> p q k h", q=3, k=3)
    nc.vector.tensor_tensor(
        out=pv, in0=nv,
        in1=rden[:].rearrange("p (q h) -> p q h", q=3)[:, :, None, :].broadcast_to((128, 3, 3, 4)),
        op=OP.mult,
    )

    if stage == 6:
        return _dbg(nc, OUT, B0, prod[:, 0:512].bitcast(F32))

    # ---- mix: o_q = v0 + p1*(v1-v0) + p2*(v2-v0) ----
    d1 = sbs.tile([128, 256], BF16, tag="d1")
    d2 = sbs.tile([128, 256], BF16, tag="d2")
    nc.vector.tensor_tensor(out=d1[:], in0=qv[:, 1, 512:768], in1=qv[:, 0, 512:768], op=OP.subtract)
    nc.vector.tensor_tensor(out=d2[:], in0=qv[:, 2, 512:768], in1=qv[:, 0, 512:768], op=OP.subtract)
    t1 = sb.tile([128, 768], BF16, tag="t1")
    t2 = sb.tile([128, 768], BF16, tag="t2")
    for dd, tt_, k in ((d1, t1, 1), (d2, t2, 2)):
        nc.vector.tensor_tensor(
            out=tt_[:].rearrange("p (q h e) -> p q h e", q=3, h=4),
            in0=dd[:, None, :].broadcast_to((128, 3, 256)).rearrange("p q (h e) -> p q h e", h=4),
            in1=pv[:, :, k, :, None].broadcast_to((128, 3, 4, 64)),
            op=OP.mult,
        )
    o = sb.tile([128, 768], BF16, tag="o")
    nc.vector.tensor_tensor(out=o[:], in0=t1[:], in1=t2[:], op=OP.add)
    nc.vector.tensor_tensor(
        out=o[:].rearrange("p (q f) -> p q f", q=3),
        in0=o[:].rearrange("p (q f) -> p q f", q=3),
        in1=qv[:, 0:1, 512:768].broadcast_to((128, 3, 256)),
        op=OP.add,
    )

    if stage == 7:
        return _dbg(nc, OUT, B0, o[:, 0:512].bitcast(F32))

    # ---- o -> fm; out-proj; x1 = tokens'' + o @ out_w.T ----
    ofm = sb.tile([128, 768], BF16, tag="ofm")
    for c in range(2):
        ps = psB.tile([128, 384], BF16, tag="psB")
        for q in range(3):
            nc.tensor.transpose(
                ps[:, q * 128 : q * 128 + 128],
                o[:, q * 256 + c * 128 : q * 256 + c * 128 + 128],
                idb,
            )
        nc.vector.tensor_copy(ofm[:, c * 384 : (c + 1) * 384], ps[:])
    x1f = sb.tile([128, 768], BF16, tag="x1f")
    for c in range(2):
        ps = psA.tile([128, 384], F32, tag="psA")
        for kc in range(2):
            mm(ps[:],
               C["c_outwt"][:, kc * 256 + c * 128 : kc * 256 + c * 128 + 128],
               ofm[:, kc * 384 : (kc + 1) * 384],
               start=(kc == 0), stop=(kc == 1))
        nc.vector.tensor_tensor(out=x1f[:, c * 384 : (c + 1) * 384], in0=ps[:],
                                in1=tok2[:, c * 384 : (c + 1) * 384], op=OP.add)

    if stage == 8:
        return _dbg(nc, OUT, B0, x1f[:, 0:512].bitcast(F32))

    # ---- LN1 (row-major) ----
    n1rm = _layernorm_rm(nc, tc, sb, sbs, psQ, psA, x1f, idb, "1")

    if stage == 9:
        return _dbg(nc, OUT, B0, n1rm[:, 0:512].bitcast(F32))

    # ---- n1 -> fm ----
    n1f = sb.tile([128, 768], BF16, tag="n1f")
    for c in range(2):
        ps = psB.tile([128, 384], BF16, tag="psB")
        for s in range(3):
            nc.tensor.transpose(
                ps[:, s * 128 : s * 128 + 128],
                n1rm[:, s * 256 + c * 128 : s * 256 + c * 128 + 128],
                idb,
            )
        nc.vector.tensor_copy(n1f[:, c * 384 : (c + 1) * 384], ps[:])

    if stage == 10:
        return _dbg(nc, OUT, B0, n1f[:, 0:512].bitcast(F32))

    # ---- FFN ----
    f1 = sb.tile([128, 1536], BF16, tag="f1")
    for oc in range(4):
        ps = psA.tile([128, 384], F32, tag="psA")
        for kc in range(2):
            mm(ps[:],
               C["c_ffn1t"][:, kc * 512 + oc * 128 : kc * 512 + oc * 128 + 128],
               n1f[:, kc * 384 : (kc + 1) * 384],
               start=(kc == 0), stop=(kc == 1))
        nc.scalar.activation(f1[:, oc * 384 : (oc + 1) * 384], ps[:], GELU,
                             bias=C["c_ffn1b"][:, oc : oc + 1])
    x2f = sb.tile([128, 768], BF16, tag="x2f")
    for c in range(2):
        ps = psA.tile([128, 384], F32, tag="psA")
        for kc in range(4):
            mm(ps[:],
               C["c_ffn2t"][:, kc * 256 + c * 128 : kc * 256 + c * 128 + 128],
               f1[:, kc * 384 : (kc + 1) * 384],
               start=(kc == 0), stop=False)
        mm(ps[:], C["c_g1d"][:, c * 128 : (c + 1) * 128],
           n1f[:, c * 384 : (c + 1) * 384], start=False, stop=True)
        nc.scalar.activation(x2f[:, c * 384 : (c + 1) * 384], ps[:], AF.Identity,
                             bias=C["c_x2b"][:, c : c + 1])

    if stage == 11:
        return _dbg(nc, OUT, B0, x2f[:, 0:512].bitcast(F32))

    # ---- LN2 (row-major) ----
    n2rm = _layernorm_rm(nc, tc, sb, sbs, psQ, psA, x2f, idb, "2")

    if stage == 12:
        return _dbg(nc, OUT, B0, n2rm[:, 0:512].bitcast(F32))

    # ---- masked mean pool + ln2 affine ----
    acc = pout.tile([128, 256], F32, tag="acc")
    nc.vector.scalar_tensor_tensor(
        out=acc[:], in0=n2rm[:, 0:256], scalar=wm[:, 0:1], in1=n2rm[:, 0:256],
        op0=OP.mult, op1=OP.bypass,
    )
    for s in (1, 2):
        nc.vector.scalar_tensor_tensor(
            out=acc[:], in0=n2rm[:, s * 256 : (s + 1) * 256],
            scalar=wm[:, s : s + 1], in1=acc[:], op0=OP.mult, op1=OP.add,
        )
    outt = pout.tile([128, 256], F32, tag="outt")
    nc.vector.tensor_tensor(out=outt[:], in0=acc[:], in1=C["c_g2b"], op=OP.mult)
    nc.vector.tensor_tensor(out=outt[:], in0=outt[:], in1=C["c_b2b"], op=OP.add)
    nc.sync.dma_start(OUT[B0 : B0 + NB, :], outt[:])


LN_STAGE = None


def _rsqrt(nc, sbs, v, n, tag):
    """y = 1/sqrt(v) elementwise on a [128, n] f32 tile, DVE only."""
    yi = sbs.tile([128, n], I32, tag=tag + "_i")
    nc.vector.tensor_scalar(out=yi[:], in0=v[:].bitcast(I32), scalar1=1,
                            scalar2=0xFFFFFFFF, op0=OP.logical_shift_right,
                            op1=OP.bitwise_xor)
    nc.vector.tensor_scalar(out=yi[:], in0=yi[:], scalar1=RSQRT_MAGIC + 1,
                            scalar2=None, op0=OP.add)
    y = yi[:].bitcast(F32)
    t1 = sbs.tile([128, n], F32, tag=tag + "_t1")
    t2 = sbs.tile([128, n], F32, tag=tag + "_t2")
    for _ in range(2):
        nc.vector.tensor_tensor(out=t1[:], in0=y, in1=y, op=OP.mult)
        nc.vector.tensor_tensor(out=t2[:], in0=v[:], in1=t1[:], op=OP.mult)
        nc.vector.tensor_scalar(out=t2[:], in0=t2[:], scalar1=-0.5, scalar2=1.5,
                                op0=OP.mult, op1=OP.add)
        nc.vector.tensor_tensor(out=yi[:].bitcast(F32), in0=y, in1=t2[:], op=OP.mult)
    return yi[:].bitcast(F32)


def _layernorm_rm(nc, tc, sb, sbs, psQ, psA, xf, idb, suffix):
    """xf: (128, 2*384) bf16 feature-major -> normalized row-major (128, 768)
    bf16 (no affine)."""
    psr = psQ.tile([128, 768], BF16, tag="psQ")
    for c in range(2):
        for s in range(3):
            nc.tensor.transpose(
                psr[:, s * 256 + c * 128 : s * 256 + c * 128 + 128],
                xf[:, c * 384 + s * 128 : c * 384 + s * 128 + 128],
                idb,
            )
    xrm = sb.tile([128, 768], BF16, tag="xrm" + suffix)
    sums = sbs.tile([128, 3], F32, tag="sums" + suffix)
    sqs = sbs.tile([128, 3], F32, tag="sqs" + suffix)
    scr = sb.tile([128, 768], BF16, tag="scr" + suffix)
    for s in range(3):
        nc.scalar.activation(
            xrm[:, s * 256 : (s + 1) * 256], psr[:, s * 256 : (s + 1) * 256],
            AF.Copy, accum_out=sums[:, s : s + 1],
        )
        nc.scalar.activation(
            scr[:, s * 256 : (s + 1) * 256], xrm[:, s * 256 : (s + 1) * 256],
            AF.Square, accum_out=sqs[:, s : s + 1],
        )
    if LN_STAGE in (81, 815, 82):
        return xrm
    mean = sbs.tile([128, 3], F32, tag="mean" + suffix)
    nc.vector.tensor_scalar_mul(mean[:], sums[:], 1.0 / 256.0)
    ex2 = sbs.tile([128, 3], F32, tag="ex2" + suffix)
    nc.vector.tensor_scalar(out=ex2[:], in0=sqs[:], scalar1=1.0 / 256.0,
                            scalar2=EPS, op0=OP.mult, op1=OP.add)
    m2 = sbs.tile([128, 3], F32, tag="m2" + suffix)
    nc.vector.tensor_tensor(out=m2[:], in0=mean[:], in1=mean[:], op=OP.mult)
    var = sbs.tile([128, 3], F32, tag="var" + suffix)
    nc.vector.tensor_tensor(out=var[:], in0=ex2[:], in1=m2[:], op=OP.subtract)
    # rstd = 1/sqrt(var) via magic-constant + 2 Newton steps (no Sqrt table)
    rstd = _rsqrt(nc, sbs, var, 3, "rstd" + suffix)
    mb = sbs.tile([128, 3], F32, tag="mb" + suffix)
    nc.vector.scalar_tensor_tensor(out=mb[:], in0=mean[:], scalar=-1.0,
                                   in1=rstd[:], op0=OP.mult, op1=OP.mult)
    if LN_STAGE == 83:
        return xrm
    nrm = sb.tile([128, 768], BF16, tag="nrm" + suffix)
    for s in range(3):
        nc.scalar.activation(
            nrm[:, s * 256 : (s + 1) * 256], xrm[:, s * 256 : (s + 1) * 256],
            AF.Identity, bias=mb[:, s : s + 1], scale=rstd[:, s : s + 1],
        )
    return nrm


_CACHE: dict = {}


def _get_nc(bl: int) -> bass.Bass:
    if bl not in _CACHE:
        _CACHE[bl] = build(bl)
    return _CACHE[bl]


def kernel(**inputs) -> np.ndarray:
    consts = host_consts(inputs)
    nc = _get_nc(B // NCORES)
    in_maps = [host_inputs(i, inputs, consts) for i in range(NCORES)]
    res = run_bass_kernel_spmd(nc, in_maps, core_ids=list(range(NCORES)))
    return np.concatenate([r["out"] for r in res.results], axis=0)

